# revision 4
# baseline (speedup 1.0000x reference)
"""LoRA linear kernel for Trainium2 (8 NeuronCores, SPMD data-parallel).

Computes out = x @ (A @ B) for
    x: [4, 2048, 4096] f32, A: [4096, 16] f32, B: [16, 4096] f32
by reassociating to (x @ A) @ B  (4.3 GFLOP instead of 274 GFLOP).

Sharding: x is split row-wise (batch*seq = 8192 rows -> 1024 rows/core).
Each core's shard is pre-transposed on the host to xT [4096, 1024] so the
contraction dim (d_in) lands on SBUF partitions naturally:

  stage 1:  tT[16, n]   = sum_c A_c[128,16].T @ xT_c[128, n]   (PSUM accum)
  stage 2:  out[128, d] = tT[:, rb].T @ B[16, d]               (single MM)

A and B are replicated to every core. No collectives.
"""

import numpy as np

import concourse.bass as bass
import concourse.bacc as bacc
import concourse.mybir as mybir
from concourse.tile import TileContext
from concourse.bass_utils import run_bass_kernel_spmd

N_CORES = 8
BATCH, SEQ, D_IN, D_OUT, R = 4, 2048, 4096, 4096, 16
ROWS = BATCH * SEQ              # 8192
RPC = ROWS // N_CORES           # 1024 rows per core
KC = D_IN // 128                # 32 contraction chunks of 128
RCHUNK = 256                    # rows per stage-1 chunk
NCH = RPC // RCHUNK             # 4 chunks per core
DC = 512                        # d_out columns per stage-2 matmul (PSUM bank)
NDC = D_OUT // DC               # 8

F32 = mybir.dt.float32

_cache = {}


def _build(mm_dtype=F32):
    nc = bacc.Bacc("TRN2", target_bir_lowering=False)
    xT = nc.dram_tensor("xT", [D_IN, RPC], mm_dtype, kind="ExternalInput")
    A = nc.dram_tensor("A", [D_IN, R], mm_dtype, kind="ExternalInput")
    Bw = nc.dram_tensor("Bw", [R, D_OUT], mm_dtype, kind="ExternalInput")
    out = nc.dram_tensor("out", [RPC, D_OUT], F32, kind="ExternalOutput")

    xT3 = xT.rearrange("(c p) n -> p c n", p=128)   # [128, KC, RPC]
    A3 = A.rearrange("(c p) r -> p c r", p=128)     # [128, KC, R]

    with TileContext(nc) as tc:
        with (
            tc.tile_pool(name="consts", bufs=1) as cpool,
            tc.tile_pool(name="xin", bufs=2) as xpool,
            tc.tile_pool(name="tbuf", bufs=2) as tpool,
            tc.tile_pool(name="obuf", bufs=2) as opool,
            tc.tile_pool(name="pt", bufs=2, space="PSUM") as ptpool,
            tc.tile_pool(name="po", bufs=4, space="PSUM") as popool,
        ):
            a_tile = cpool.tile([128, KC, R], mm_dtype)
            nc.sync.dma_start(out=a_tile[:], in_=A3[:, :, :])
            b_tile = cpool.tile([R, D_OUT], mm_dtype)
            nc.sync.dma_start(out=b_tile[:], in_=Bw[:, :])

            for rc in range(NCH):
                n0 = rc * RCHUNK
                xt = xpool.tile([128, KC, RCHUNK], mm_dtype)
                nc.sync.dma_start(out=xt[:], in_=xT3[:, :, n0:n0 + RCHUNK])

                # stage 1: tT [16, RCHUNK] = (x_chunk @ A).T
                pt = ptpool.tile([R, RCHUNK], F32)
                for c in range(KC):
                    nc.tensor.matmul(
                        pt[:],
                        a_tile[:, c, :],
                        xt[:, c, :],
                        start=(c == 0),
                        stop=(c == KC - 1),
                    )
                tT = tpool.tile([R, RCHUNK], mm_dtype)
                nc.vector.tensor_copy(tT[:], pt[:])

                # stage 2: out rows = tT.T @ B, one 128-row block at a time
                for rb in range(RCHUNK // 128):
                    osb = opool.tile([128, D_OUT], F32)
                    for dc in range(NDC):
                        po = popool.tile([128, DC], F32)
                        nc.tensor.matmul(
                            po[:],
                            tT[:, rb * 128:(rb + 1) * 128],
                            b_tile[:, dc * DC:(dc + 1) * DC],
                            start=True,
                            stop=True,
                        )
                        nc.vector.tensor_copy(osb[:, dc * DC:(dc + 1) * DC], po[:])
                    row0 = n0 + rb * 128
                    nc.sync.dma_start(out=out[row0:row0 + 128, :], in_=osb[:])
    nc.compile()
    return nc


def _get_nc(mm_dtype=F32):
    key = str(mm_dtype)
    if key not in _cache:
        _cache[key] = _build(mm_dtype)
    return _cache[key]


def kernel(x, A, B, trace=False, mm_dtype=F32):
    x = np.asarray(x, dtype=np.float32)
    A = np.ascontiguousarray(np.asarray(A, dtype=np.float32))
    B = np.ascontiguousarray(np.asarray(B, dtype=np.float32))
    xf = x.reshape(ROWS, D_IN)

    nc = _get_nc(mm_dtype)
    in_maps = []
    for i in range(N_CORES):
        xs = xf[i * RPC:(i + 1) * RPC]                 # [1024, 4096]
        xT = np.ascontiguousarray(xs.T)                # [4096, 1024]
        in_maps.append({"xT": xT, "A": A, "Bw": B})

    res = run_bass_kernel_spmd(nc, in_maps, list(range(N_CORES)), trace=trace)
    outs = [res.results[i]["out"] for i in range(N_CORES)]
    full = np.concatenate(outs, axis=0).reshape(BATCH, SEQ, D_OUT)
    if trace:
        kernel.last_exec_time_ns = res.exec_time_ns
        kernel.last_results = res
    return full
